# revision 13
# baseline (speedup 1.0000x reference)
"""Trainium2 Bass kernel for nn_MultiHeadAttnC (QANet-style self-attention).

Reference computation (per batch b):
    memory = w_mem @ queries[b]          # [2D, L]  (pointwise conv)
    query  = w_query @ queries[b]        # [D, L]
    K, V   = heads of memory             # H=8 heads, DH=16
    Q      = heads of query * DH^-0.5
    S      = Q @ K^T  (masked over kv)   # [H, L, L]
    out[b] = softmax(S) @ V  -> recombined to [D, L]

Strategy (v6):
  - Data parallel: batch b -> NeuronCore b. Weights replicated. No collectives.
  - The exp (27M/core, PSUM-resident so only ACT+DVE can touch it) is the
    roofline (~117us at the two engines' combined ~1.8 cols/ns). The whole
    design keeps ACT/DVE saturated with [128, 1024] exp instructions:
      - q tiles are QT=256 wide; one CHUNK tile = [128 kv, 4 heads x 256 q]
        = 2 PSUM banks, produced by ONE 4-way row-banded matmul quad
        (~256 column-beats when merged) and consumed by ONE exp instr.
      - chunk tiles rotate through a SEVEN-bank PSUM ring (manual slot
        rotation over a [128, 3584] region; a pair lands on banks (6,0)
        once per 7 allocations and is handled as two 512 halves). Depth
        ~3.5 chunk-slots hides the S-production latency + scheduler
        noise that a 3-duo ring exposed every tile.
      - both stream accumulators share the 8th bank ([128, 256] halves):
        AV quads write partition-disjoint 17-row bands, col-disjoint
        stream halves; one [128, 256] drain per stream.
      - AV quads are batched into two half-stream runs; the 2nd run,
        drain, and epilogue are deferred into the NEXT stream's chunk
        loop (flush at chunk 5/8/11), so on the in-order engine queues
        they sit behind many exps and never stall on the AV tail.
      - epilogue per stream: ONE stride+broadcast DMA lifts the 4
        denominator rows into band layout, full-width reciprocal (DVE),
        multiply on GpSimd (SBUF-only engine), 4 band DMAs to DRAM.
  - exp split between ACT (native Exp) and DVE (single-op Schraudolph:
    i16 = round(x*A + B) == bf16 bits of exp(x)) by a greedy time ledger
    with HW-calibrated costs. Masked kv compacted host-side (exact).
"""

import numpy as np
from contextlib import ExitStack

import concourse.bass as bass
import concourse.tile as tile
from concourse import bacc, mybir
from concourse import bass_utils

B, D, L, H, DH = 8, 128, 2048, 8, 16
f32 = mybir.dt.float32
bf16 = mybir.dt.bfloat16
i16 = mybir.dt.int16
f32r = mybir.dt.float32r
IN_DT = f32r
QT = 512             # q columns per stream tile
NJQ = L // QT        # 4
NXP = 2              # weight spread planes (4 head-groups each)
SW = 512             # PSUM ring slot width (one bank of f32)

# Schraudolph exp constants for round-to-nearest f32->i16 conversion:
# bf16_bits(exp(x)) ~= round(x * 2^7*log2(e) + (127*2^7 - 5.6))
EXP_A = 184.6649652337873
EXP_B = 16250.4

_program_cache: dict = {}


def _body(ctx, tc, qf_d, qkv_d, wq_d, wk_d, wv_d, val_d, out_d, n_kv, compact):
    nc = tc.nc
    Lkv = n_kv * 128
    Exp = mybir.ActivationFunctionType.Exp
    Copy = mybir.ActivationFunctionType.Copy
    mult, add = mybir.AluOpType.mult, mybir.AluOpType.add
    NX = NXP

    consts = ctx.enter_context(tc.tile_pool(name="consts", bufs=1))

    # ---- input DMAs: weights + first blocks first. SP and ACT are the two
    # HW-DGE engines (gpsimd DMA is slow software DGE) - at startup ACT is
    # idle, so spread the input over all three descriptor streams ----
    wq = consts.tile([D, NX, D], IN_DT, tag="wq")
    wk = consts.tile([D, NX, D], IN_DT, tag="wk")
    wv = consts.tile([D, D], IN_DT, tag="wv")
    nc.sync.dma_start(out=wk[:, 0, :], in_=wk_d[0])
    nc.scalar.dma_start(out=wq[:, 0, :], in_=wq_d[0])
    nc.sync.dma_start(out=wk[:, 1, :], in_=wk_d[1])
    nc.scalar.dma_start(out=wq[:, 1, :], in_=wq_d[1])
    nc.gpsimd.dma_start(out=wv, in_=wv_d)
    qkv = consts.tile([D, Lkv], IN_DT, tag="qkv")
    qf = consts.tile([D, L], IN_DT, tag="qf")
    qs_list = [(qkv, qkv_d, c, min(512, Lkv - c))
               for c in range(0, Lkv, 512)]
    qf_list = [(qf, qf_d, j * 512, 512) for j in range(L // 512)]
    # interleave kv/q blocks so both proj streams can start early
    order = []
    for a, b in zip(qs_list, qf_list + [None] * 9):
        order.append(a)
        if b:
            order.append(b)
    engs = [nc.sync, nc.scalar, nc.gpsimd]
    for i, (t, d, c, n) in enumerate(order):
        engs[i % 3].dma_start(out=t[:, c:c + n], in_=d[:, c:c + n])

    q_sp = consts.tile([D, NX, L], bf16, tag="q_sp")
    k_sp = consts.tile([D, NX, Lkv], bf16, tag="k_sp")
    v_sb = consts.tile([128, n_kv, H, DH + 1], bf16, tag="v_sb")

    if compact:
        # validity (pre-broadcast per head host-side) -> ones column of v_sb
        # via DVE strided write (DMA would clobber neighbors: 2-byte elems)
        val16 = consts.tile([128, n_kv * H], bf16, tag="val16")
        nc.sync.dma_start(out=val16, in_=val_d)
        dst = bass.AP(tensor=v_sb.tensor, offset=v_sb.offset + DH,
                      ap=[[n_kv * H * (DH + 1), 128], [DH + 1, n_kv * H]])
        nc.vector.tensor_copy(out=dst, in_=val16)
    else:
        val = consts.tile([128, n_kv], f32, tag="val")
        nc.gpsimd.dma_start(out=val, in_=val_d)
        ones8 = consts.tile([128, 8], f32, tag="ones8")
        nc.gpsimd.memset(ones8, 1.0)

    # ---- PSUM: 7-bank chunk-tile ring (manual rotation) + 1 acc bank ----
    psum = ctx.enter_context(tc.tile_pool(name="psum", bufs=1, space="PSUM"))
    ring_t = psum.tile([128, 7 * SW], f32, tag="ring", name="ring")
    acc_t = psum.tile([128, SW], f32, tag="acc", name="acc")

    slot = [0]

    def alloc(k):
        """Allocate k ring slots (k=1 or 2). Returns [(col, width), ...]
        segments; a k=2 allocation wraps (6,0) into two 512 halves."""
        s = slot[0]
        if k == 1:
            slot[0] = (s + 1) % 7
            return [(s * SW, SW)]
        if s == 6:
            slot[0] = 1
            return [(6 * SW, SW), (0, SW)]
        slot[0] = (s + 2) % 7
        return [(s * SW, 2 * SW)]

    # ---- HAM warmup + ACT exp-table prime ----
    warm_in = consts.tile([128, 512], bf16, tag="warm_in")
    nc.gpsimd.memset(warm_in, 0.0)
    for i in range(3):
        nc.tensor.matmul(acc_t, lhsT=warm_in[:, 0:128], rhs=warm_in,
                         start=True, stop=True, skip_group_check=True)
    p_warm = consts.tile([128, 128], bf16, tag="p_warm")
    nc.scalar.activation(out=p_warm, in_=warm_in[:, 0:128], func=Exp)

    # ---- ACT/DVE time ledger (HW-calibrated: ACT ~n/1.2+210ns,
    # DVE ~n/0.96+135ns per instruction) ----
    eng_t = {"act": 0.0, "dve": 0.0}

    def pick(n):
        c_act = eng_t["act"] + n / 1.2 + 210
        c_dve = eng_t["dve"] + n / 0.96 + 135
        if c_act <= c_dve:
            eng_t["act"] = c_act
            return "act"
        eng_t["dve"] = c_dve
        return "dve"

    def evac(dst_ap, src_ap, n):
        if pick(n) == "act":
            nc.scalar.activation(out=dst_ap, in_=src_ap, func=Copy)
        else:
            nc.vector.tensor_copy(out=dst_ap, in_=src_ap)

    def proj_k(X):
        for col in range(0, Lkv, 512):
            n = min(512, Lkv - col)
            (c0, _), = alloc(1)
            nc.tensor.matmul(ring_t[:, c0:c0 + n], lhsT=wk[:, X, :],
                             rhs=qkv[:, col:col + n], start=True, stop=True)
            evac(k_sp[:, X, col:col + n], ring_t[:, c0:c0 + n], n)

    def proj_q(X):
        for col in range(0, L, 512):
            (c0, _), = alloc(1)
            nc.tensor.matmul(ring_t[:, c0:c0 + 512], lhsT=wq[:, X, :],
                             rhs=qf[:, col:col + 512], start=True, stop=True)
            evac(q_sp[:, X, col:col + 512], ring_t[:, c0:c0 + 512], 512)

    def proj_v():
        for c0 in range(0, n_kv, 4):
            nb = min(4, n_kv - c0)
            (b0, _), = alloc(1)
            for ci in range(nb):
                c = c0 + ci
                nc.tensor.matmul(ring_t[:, b0 + ci * D:b0 + (ci + 1) * D],
                                 lhsT=qkv[:, c * 128:(c + 1) * 128],
                                 rhs=wv, start=True, stop=True)
            for ci in range(nb):
                c = c0 + ci
                src = ring_t[:, b0 + ci * D:b0 + (ci + 1) * D].rearrange(
                    "p (h x) -> p h x", x=DH)
                if compact:
                    evac(v_sb[:, c, :, 0:DH], src, D)
                else:
                    nc.vector.tensor_scalar_mul(v_sb[:, c, :, 0:DH], src,
                                                val[:, c:c + 1])
                    nc.vector.tensor_scalar_mul(
                        v_sb[:, c, :, DH:DH + 1],
                        ones8.rearrange("p (h x) -> p h x", x=1),
                        val[:, c:c + 1])

    # ---- attention ----
    p_act = ctx.enter_context(tc.tile_pool(name="p_act", bufs=15))
    p_dve = ctx.enter_context(tc.tile_pool(name="p_dve", bufs=14))
    a_pool = ctx.enter_context(tc.tile_pool(name="a_pool", bufs=3))
    m_pool = ctx.enter_context(tc.tile_pool(name="m_pool", bufs=2))

    a_sbs = {}   # si -> SBUF a_sb tile (until epilogue)
    pend = []    # deferred emission: (at_chunk, closure), flushed inside
                 # the NEXT stream's chunk loop

    def stream(si, jq, X, last=False):
        """One (jq, X) stream: 4 heads x QT q columns over all kv chunks.
        Chunk c: two S pairs (one head per ring bank: two row-banded
        matmuls may NOT share a psum bank) -> two [128, 1024] exps ->
        two P tiles. AV quads batched; run 2 + drain + epilogue deferred.
        The single acc bank is safe with bufs=1: stream si+1's first AV
        run is emitted after drain(si) flushes (chunk 8 of si+1)."""
        qs = slice(jq * QT, (jq + 1) * QT)
        ps = {}

        def s_pair(c, j):
            ck = slice(c * 128, (c + 1) * 128)
            segs = alloc(2)
            for gi in (0, 1):
                g = 2 * j + gi
                col = (segs[0][0] + gi * SW) if len(segs) == 1 \
                    else segs[gi][0]
                nc.tensor.matmul(
                    ring_t[:, col:col + SW],
                    lhsT=k_sp[g * 32:(g + 1) * 32, X, ck],
                    rhs=q_sp[g * 32:(g + 1) * 32, X, qs],
                    start=True, stop=True, tile_position=(g * 32, 0))
            return segs

        def do_exp(c, j, segs):
            if pick(1024) == "act":
                p = p_act.tile([128, 2 * QT], bf16, tag="p")
                first = "act"
            else:
                p16 = p_dve.tile([128, 2 * QT], i16, tag="p16")
                p = p16.bitcast(bf16)
                first = "dve"
            off = 0
            for k, (c0, w) in enumerate(segs):
                eng = first if k == 0 else pick(w)
                if eng == "act":
                    nc.scalar.activation(out=p[:, off:off + w],
                                         in_=ring_t[:, c0:c0 + w], func=Exp)
                else:
                    nc.vector.tensor_scalar(
                        out=p[:, off:off + w].bitcast(i16),
                        in0=ring_t[:, c0:c0 + w],
                        scalar1=EXP_A, scalar2=EXP_B, op0=mult, op1=add)
                off += w
            ps[(c, j)] = p

        def emit_av(c):
            st, en = (c == 0), (c == n_kv - 1)
            for j in (0, 1):
                p = ps.pop((c, j))
                for gi in (0, 1):
                    g = 2 * j + gi
                    nc.tensor.matmul(
                        acc_t[g * 32:g * 32 + DH + 1, :],
                        lhsT=v_sb[:, c, 4 * X + g, :],
                        rhs=p[:, gi * QT:(gi + 1) * QT],
                        start=st, stop=en, skip_group_check=True,
                        tile_position=(0, g * 32))

        half = n_kv // 2

        def run_b():
            for c in range(half, n_kv):
                emit_av(c)

        prev = pend[:]
        pend.clear()
        sq = {j: s_pair(0, j) for j in (0, 1)}
        for c in range(n_kv):
            for j in (0, 1):
                do_exp(c, j, sq[j])
            if c + 1 < n_kv:
                sq = {j: s_pair(c + 1, j) for j in (0, 1)}
            for at, fn in prev:
                if at == c:
                    fn()
        for at, fn in prev:
            if at >= n_kv:
                fn()
        for c in range(half):
            emit_av(c)
        if last:
            run_b()
        else:
            pend.append((5, run_b))

    def drain(si, split=False):
        a_sb = a_pool.tile([128, QT], f32, tag="a", name="a")
        if split:
            h = QT // 2
            nc.scalar.activation(out=a_sb[:, 0:h], in_=acc_t[:, 0:h],
                                 func=Copy)
            nc.vector.tensor_copy(out=a_sb[:, h:QT], in_=acc_t[:, h:QT])
        else:
            evac(a_sb, acc_t, QT)
        a_sbs[si] = a_sb

    def epilogue(si, jq, X, last=False):
        qs = slice(jq * QT, (jq + 1) * QT)
        a_sb = a_sbs.pop(si)
        # denominator rows {16,48,80,112} broadcast to band layout
        rb = m_pool.tile([128, QT], f32, tag="rb", name="rb")
        nc.sync.dma_start(
            out=rb,
            in_=bass.AP(tensor=a_sb.tensor, offset=a_sb.offset + 16 * QT,
                        ap=[[32 * QT, 4], [0, 32], [1, QT]]))
        rec = m_pool.tile([128, QT], f32, tag="rec", name="rec")
        nc.vector.reciprocal_approx_fast(out=rec, in_=rb)
        eng_t["dve"] += QT / 0.96 + 135
        mout = m_pool.tile([128, QT], f32, tag="m", name="m")
        if last:
            h = QT // 2
            nc.gpsimd.tensor_mul(out=mout[:, 0:h], in0=a_sb[:, 0:h],
                                 in1=rec[:, 0:h])
            nc.vector.tensor_mul(out=mout[:, h:QT], in0=a_sb[:, h:QT],
                                 in1=rec[:, h:QT])
        else:
            nc.gpsimd.tensor_mul(out=mout, in0=a_sb, in1=rec)
        # scatter head bands g*32+i -> DRAM channel rows (4X+g)*16+i
        for g in range(4):
            h = 4 * X + g
            eng = nc.sync if g % 2 == 0 else nc.gpsimd
            eng.dma_start(out=out_d[h * DH:(h + 1) * DH, qs],
                          in_=mout[g * 32:g * 32 + DH, :])

    # ---- emission schedule ----
    proj_k(0)
    proj_q(0)
    proj_v()
    streams = [(jq, 0) for jq in range(NJQ)] + \
              [(jq, 1) for jq in range(NJQ)]
    for si, (jq, X) in enumerate(streams):
        lastr = si == len(streams) - 1
        stream(si, jq, X, last=lastr)
        if not lastr:
            pend.append((8, (lambda s: lambda: drain(s))(si)))
            pend.append((11, (lambda s, j, x: lambda: epilogue(s, j, x))(
                si, jq, X)))
        if si == 0:
            proj_k(1)
        elif si == 1:
            proj_q(1)
    drain(len(streams) - 1, split=True)
    epilogue(len(streams) - 1, *streams[-1], last=True)


def _build(n_kv: int, compact: bool) -> "bacc.Bacc":
    Lkv = n_kv * 128
    NX = NXP
    nc = bacc.Bacc("TRN2", target_bir_lowering=False, debug=False,
                   enable_asserts=True, num_devices=B)
    qf_d = nc.dram_tensor("q_full", [D, L], IN_DT, kind="ExternalInput").ap()
    qkv_d = nc.dram_tensor("q_kv", [D, Lkv], IN_DT, kind="ExternalInput").ap()
    wq_d = nc.dram_tensor("wq_sp", [NX, D, D], IN_DT, kind="ExternalInput").ap()
    wk_d = nc.dram_tensor("wk_sp", [NX, D, D], IN_DT, kind="ExternalInput").ap()
    wv_d = nc.dram_tensor("wv_t", [D, D], IN_DT, kind="ExternalInput").ap()
    val_dt = bf16 if compact else f32
    val_shape = [128, n_kv * H] if compact else [128, n_kv]
    val_d = nc.dram_tensor("valid", val_shape, val_dt,
                           kind="ExternalInput").ap()
    out_d = nc.dram_tensor("out", [D, L], f32, kind="ExternalOutput").ap()

    with tile.TileContext(nc) as tc, ExitStack() as ctx:
        _body(ctx, tc, qf_d, qkv_d, wq_d, wk_d, wv_d, val_d, out_d, n_kv,
              compact)
    nc.compile()
    return nc


def _prep_weights(w_mem: np.ndarray, w_query: np.ndarray):
    """Spread head weights into 32-row tile groups (rows 16:32 zero) across
    two planes of 4 head-groups, pre-transposed for use as matmul lhsT.
    Q gets the DH^-0.5 scale."""
    wq_sp = np.zeros((NXP, D, D), np.float32)
    wk_sp = np.zeros((NXP, D, D), np.float32)
    scale = np.float32(DH ** -0.5)
    for X in range(NXP):
        for g in range(4):
            h = 4 * X + g
            wq_sp[X][:, 32 * g:32 * g + DH] = (w_query[DH * h:DH * (h + 1), :] * scale).T
            wk_sp[X][:, 32 * g:32 * g + DH] = w_mem[DH * h:DH * (h + 1), :].T
    wv_t = np.ascontiguousarray(w_mem[D:2 * D, :].T)
    return wq_sp, wk_sp, wv_t


COMPACT_KV = True  # drop masked kv positions host-side (exact: they get a
                   # zero validity column -> contribute 0 to num and denom)


def prepare(queries: np.ndarray, mask: np.ndarray, w_mem: np.ndarray,
            w_query: np.ndarray):
    """Build (compiled program, per-core input maps)."""
    import ml_dtypes
    assert queries.shape == (B, D, L) and mask.shape == (B, L)
    maskf = mask.astype(np.float32)
    kept = [np.nonzero(maskf[b] > 0.0)[0] for b in range(B)]
    if COMPACT_KV and all(len(k) > 0 for k in kept):
        n_kv = max(1, -(-max(len(k) for k in kept) // 128))
        compact = True
    else:
        n_kv = L // 128
        kept = None
        compact = False
    Lkv = n_kv * 128

    key = (n_kv, compact)
    nc = _program_cache.get(key)
    if nc is None:
        nc = _program_cache[key] = _build(n_kv, compact)

    wq_sp, wk_sp, wv_t = _prep_weights(
        w_mem.astype(np.float32), w_query.astype(np.float32))

    in_maps = []
    for b in range(B):
        qb = np.ascontiguousarray(queries[b], dtype=np.float32)
        if kept is not None:
            idx = kept[b]
            qkv = np.zeros((D, Lkv), np.float32)
            qkv[:, :len(idx)] = qb[:, idx]
            val = np.zeros(Lkv, np.float32)
            val[:len(idx)] = 1.0
        else:
            qkv = qb
            val = maskf[b]
        valT = np.ascontiguousarray(val.reshape(n_kv, 128).T)
        in_maps.append({
            "q_full": qb,
            "q_kv": np.ascontiguousarray(qkv),
            "wq_sp": wq_sp,
            "wk_sp": wk_sp,
            "wv_t": wv_t,
            "valid": (np.ascontiguousarray(np.repeat(valT, H, axis=1))
                      .astype(ml_dtypes.bfloat16) if compact else valT),
        })
    return nc, in_maps


def kernel(queries: np.ndarray, mask: np.ndarray, w_mem: np.ndarray,
           w_query: np.ndarray) -> np.ndarray:
    nc, in_maps = prepare(queries, mask, w_mem, w_query)
    res = bass_utils.run_bass_kernel_spmd(nc, in_maps, core_ids=list(range(B)))
    return np.stack([res.results[b]["out"] for b in range(B)]).astype(np.float32)


# revision 14
# speedup vs baseline: 1.8313x; 1.8313x over previous
"""Trainium2 Bass kernel for nn_MultiHeadAttnC (QANet-style self-attention).

Reference computation (per batch b):
    memory = w_mem @ queries[b]          # [2D, L]  (pointwise conv)
    query  = w_query @ queries[b]        # [D, L]
    K, V   = heads of memory             # H=8 heads, DH=16
    Q      = heads of query * DH^-0.5
    S      = Q @ K^T  (masked over kv)   # [H, L, L]
    out[b] = softmax(S) @ V  -> recombined to [D, L]

Strategy (v6):
  - Data parallel: batch b -> NeuronCore b. Weights replicated. No collectives.
  - The exp (27M/core, PSUM-resident so only ACT+DVE can touch it) is the
    roofline (~117us at the two engines' combined ~1.8 cols/ns). The whole
    design keeps ACT/DVE saturated with [128, 1024] exp instructions:
      - q tiles are QT=256 wide; one CHUNK tile = [128 kv, 4 heads x 256 q]
        = 2 PSUM banks, produced by ONE 4-way row-banded matmul quad
        (~256 column-beats when merged) and consumed by ONE exp instr.
      - chunk tiles rotate through a SEVEN-bank PSUM ring (manual slot
        rotation over a [128, 3584] region; a pair lands on banks (6,0)
        once per 7 allocations and is handled as two 512 halves). Depth
        ~3.5 chunk-slots hides the S-production latency + scheduler
        noise that a 3-duo ring exposed every tile.
      - both stream accumulators share the 8th bank ([128, 256] halves):
        AV quads write partition-disjoint 17-row bands, col-disjoint
        stream halves; one [128, 256] drain per stream.
      - AV quads are batched into two half-stream runs; the 2nd run,
        drain, and epilogue are deferred into the NEXT stream's chunk
        loop (flush at chunk 5/8/11), so on the in-order engine queues
        they sit behind many exps and never stall on the AV tail.
      - epilogue per stream: ONE stride+broadcast DMA lifts the 4
        denominator rows into band layout, full-width reciprocal (DVE),
        multiply on GpSimd (SBUF-only engine), 4 band DMAs to DRAM.
  - exp split between ACT (native Exp) and DVE (single-op Schraudolph:
    i16 = round(x*A + B) == bf16 bits of exp(x)) by a greedy time ledger
    with HW-calibrated costs. Masked kv compacted host-side (exact).
"""

import numpy as np
from contextlib import ExitStack

import concourse.bass as bass
import concourse.tile as tile
from concourse import bacc, mybir
from concourse import bass_utils

B, D, L, H, DH = 8, 128, 2048, 8, 16
f32 = mybir.dt.float32
bf16 = mybir.dt.bfloat16
i16 = mybir.dt.int16
f32r = mybir.dt.float32r
IN_DT = f32r
QT = 512             # q columns per stream tile
NJQ = L // QT        # 4
NXP = 2              # weight spread planes (4 head-groups each)
SW = 512             # PSUM ring slot width (one bank of f32)

# Schraudolph exp constants for round-to-nearest f32->i16 conversion:
# bf16_bits(exp(x)) ~= round(x * 2^7*log2(e) + (127*2^7 - 5.6))
EXP_A = 184.6649652337873
EXP_B = 16250.4

_program_cache: dict = {}


def _body(ctx, tc, qf_d, qkv_d, wq_d, wk_d, wv_d, val_d, out_d, n_kv, compact):
    nc = tc.nc
    Lkv = n_kv * 128
    Exp = mybir.ActivationFunctionType.Exp
    Copy = mybir.ActivationFunctionType.Copy
    mult, add = mybir.AluOpType.mult, mybir.AluOpType.add
    NX = NXP

    consts = ctx.enter_context(tc.tile_pool(name="consts", bufs=1))

    # ---- input DMAs: weights + first blocks first. SP and ACT are the two
    # HW-DGE engines (gpsimd DMA is slow software DGE) - at startup ACT is
    # idle, so spread the input over all three descriptor streams ----
    wq = consts.tile([D, NX, D], IN_DT, tag="wq")
    wk = consts.tile([D, NX, D], IN_DT, tag="wk")
    wv = consts.tile([D, D], IN_DT, tag="wv")
    nc.sync.dma_start(out=wk[:, 0, :], in_=wk_d[0])
    nc.scalar.dma_start(out=wq[:, 0, :], in_=wq_d[0])
    nc.sync.dma_start(out=wk[:, 1, :], in_=wk_d[1])
    nc.scalar.dma_start(out=wq[:, 1, :], in_=wq_d[1])
    nc.gpsimd.dma_start(out=wv, in_=wv_d)
    qkv = consts.tile([D, Lkv], IN_DT, tag="qkv")
    qf = consts.tile([D, L], IN_DT, tag="qf")
    qs_list = [(qkv, qkv_d, c, min(512, Lkv - c))
               for c in range(0, Lkv, 512)]
    qf_list = [(qf, qf_d, j * 512, 512) for j in range(L // 512)]
    # interleave kv/q blocks so both proj streams can start early
    order = []
    for a, b in zip(qs_list, qf_list + [None] * 9):
        order.append(a)
        if b:
            order.append(b)
    engs = [nc.sync, nc.scalar, nc.gpsimd]
    for i, (t, d, c, n) in enumerate(order):
        engs[i % 3].dma_start(out=t[:, c:c + n], in_=d[:, c:c + n])

    q_sp = consts.tile([D, NX, L], bf16, tag="q_sp")
    k_sp = consts.tile([D, NX, Lkv], bf16, tag="k_sp")
    v_sb = consts.tile([128, n_kv, H, DH + 1], bf16, tag="v_sb")

    if compact:
        # validity (pre-broadcast per head host-side) -> ones column of v_sb
        # via DVE strided write (DMA would clobber neighbors: 2-byte elems)
        val16 = consts.tile([128, n_kv * H], bf16, tag="val16")
        nc.sync.dma_start(out=val16, in_=val_d)
        dst = bass.AP(tensor=v_sb.tensor, offset=v_sb.offset + DH,
                      ap=[[n_kv * H * (DH + 1), 128], [DH + 1, n_kv * H]])
        nc.vector.tensor_copy(out=dst, in_=val16)
    else:
        val = consts.tile([128, n_kv], f32, tag="val")
        nc.gpsimd.dma_start(out=val, in_=val_d)
        ones8 = consts.tile([128, 8], f32, tag="ones8")
        nc.gpsimd.memset(ones8, 1.0)

    # ---- PSUM: ring of SEVEN independent 1-bank tiles (so Tile tracks
    # each slot's deps separately) + 1 acc bank ----
    psum = ctx.enter_context(tc.tile_pool(name="psum", bufs=1, space="PSUM"))
    ring_ts = [psum.tile([128, SW], f32, tag=f"r{i}", name=f"r{i}")
               for i in range(7)]
    acc_t = psum.tile([128, SW], f32, tag="acc", name="acc")

    slot = [0]

    def alloc(k):
        """Allocate k consecutive ring slots; returns the tile objects."""
        s = slot[0]
        slot[0] = (s + k) % 7
        return [ring_ts[(s + i) % 7] for i in range(k)]

    # ---- HAM warmup + ACT exp-table prime ----
    warm_in = consts.tile([128, 512], bf16, tag="warm_in")
    nc.gpsimd.memset(warm_in, 0.0)
    for i in range(3):
        nc.tensor.matmul(acc_t, lhsT=warm_in[:, 0:128], rhs=warm_in,
                         start=True, stop=True, skip_group_check=True)
    p_warm = consts.tile([128, 128], bf16, tag="p_warm")
    nc.scalar.activation(out=p_warm, in_=warm_in[:, 0:128], func=Exp)

    # ---- ACT/DVE time ledger (HW-calibrated: ACT ~n/1.2+210ns,
    # DVE ~n/0.96+135ns per instruction) ----
    eng_t = {"act": 0.0, "dve": 0.0}

    def pick(n):
        c_act = eng_t["act"] + n / 1.2 + 210
        c_dve = eng_t["dve"] + n / 0.96 + 135
        if c_act <= c_dve:
            eng_t["act"] = c_act
            return "act"
        eng_t["dve"] = c_dve
        return "dve"

    def evac(dst_ap, src_ap, n):
        if pick(n) == "act":
            nc.scalar.activation(out=dst_ap, in_=src_ap, func=Copy)
        else:
            nc.vector.tensor_copy(out=dst_ap, in_=src_ap)

    def proj_k(X):
        for col in range(0, Lkv, 512):
            n = min(512, Lkv - col)
            t, = alloc(1)
            nc.tensor.matmul(t[:, 0:n], lhsT=wk[:, X, :],
                             rhs=qkv[:, col:col + n], start=True, stop=True)
            evac(k_sp[:, X, col:col + n], t[:, 0:n], n)

    def proj_q(X):
        for col in range(0, L, 512):
            t, = alloc(1)
            nc.tensor.matmul(t, lhsT=wq[:, X, :],
                             rhs=qf[:, col:col + 512], start=True, stop=True)
            evac(q_sp[:, X, col:col + 512], t, 512)

    def proj_v():
        for c0 in range(0, n_kv, 4):
            nb = min(4, n_kv - c0)
            t, = alloc(1)
            for ci in range(nb):
                c = c0 + ci
                nc.tensor.matmul(t[:, ci * D:(ci + 1) * D],
                                 lhsT=qkv[:, c * 128:(c + 1) * 128],
                                 rhs=wv, start=True, stop=True)
            for ci in range(nb):
                c = c0 + ci
                src = t[:, ci * D:(ci + 1) * D].rearrange(
                    "p (h x) -> p h x", x=DH)
                if compact:
                    evac(v_sb[:, c, :, 0:DH], src, D)
                else:
                    nc.vector.tensor_scalar_mul(v_sb[:, c, :, 0:DH], src,
                                                val[:, c:c + 1])
                    nc.vector.tensor_scalar_mul(
                        v_sb[:, c, :, DH:DH + 1],
                        ones8.rearrange("p (h x) -> p h x", x=1),
                        val[:, c:c + 1])

    # ---- attention ----
    p_pool = ctx.enter_context(tc.tile_pool(name="p_pool", bufs=28))
    a_pool = ctx.enter_context(tc.tile_pool(name="a_pool", bufs=3))
    m_pool = ctx.enter_context(tc.tile_pool(name="m_pool", bufs=2))

    a_sbs = {}   # si -> SBUF a_sb tile (until epilogue)
    pend = []    # deferred emission: (at_chunk, closure), flushed inside
                 # the NEXT stream's chunk loop

    def stream(si, jq, X, last=False):
        """One (jq, X) stream: 4 heads x QT q columns over all kv chunks.
        Chunk c: two S pairs (one head per ring bank: two row-banded
        matmuls may NOT share a psum bank) -> two [128, 1024] exps ->
        two P tiles. AV quads batched; run 2 + drain + epilogue deferred.
        The single acc bank is safe with bufs=1: stream si+1's first AV
        run is emitted after drain(si) flushes (chunk 8 of si+1)."""
        qs = slice(jq * QT, (jq + 1) * QT)
        ps = {}

        def s_pair(c, j):
            ck = slice(c * 128, (c + 1) * 128)
            segs = alloc(2)
            for gi in (0, 1):
                g = 2 * j + gi
                nc.tensor.matmul(
                    segs[gi],
                    lhsT=k_sp[g * 32:(g + 1) * 32, X, ck],
                    rhs=q_sp[g * 32:(g + 1) * 32, X, qs],
                    start=True, stop=True, tile_position=(g * 32, 0))
            return segs

        def do_exp(c, j, segs):
            p = p_pool.tile([128, 2 * QT], bf16, tag="p")
            for k, t in enumerate(segs):
                off = k * SW
                if pick(SW) == "act":
                    nc.scalar.activation(out=p[:, off:off + SW],
                                         in_=t, func=Exp)
                else:
                    nc.vector.tensor_scalar(
                        out=p[:, off:off + SW].bitcast(i16),
                        in0=t,
                        scalar1=EXP_A, scalar2=EXP_B, op0=mult, op1=add)
            ps[(c, j)] = p

        def emit_av(c):
            st, en = (c == 0), (c == n_kv - 1)
            for j in (0, 1):
                p = ps.pop((c, j))
                for gi in (0, 1):
                    g = 2 * j + gi
                    nc.tensor.matmul(
                        acc_t[g * 32:g * 32 + DH + 1, :],
                        lhsT=v_sb[:, c, 4 * X + g, :],
                        rhs=p[:, gi * QT:(gi + 1) * QT],
                        start=st, stop=en, skip_group_check=True,
                        tile_position=(0, g * 32))

        half = n_kv // 2

        def run_b():
            for c in range(half, n_kv):
                emit_av(c)

        prev = pend[:]
        pend.clear()
        sq = {j: s_pair(0, j) for j in (0, 1)}
        for c in range(n_kv):
            for j in (0, 1):
                do_exp(c, j, sq[j])
            if c + 1 < n_kv:
                sq = {j: s_pair(c + 1, j) for j in (0, 1)}
            for at, fn in prev:
                if at == c:
                    fn()
        for at, fn in prev:
            if at >= n_kv:
                fn()
        for c in range(half):
            emit_av(c)
        if last:
            run_b()
        else:
            pend.append((5, run_b))

    def drain(si, split=False):
        a_sb = a_pool.tile([128, QT], f32, tag="a", name="a")
        if split:
            h = QT // 2
            nc.scalar.activation(out=a_sb[:, 0:h], in_=acc_t[:, 0:h],
                                 func=Copy)
            nc.vector.tensor_copy(out=a_sb[:, h:QT], in_=acc_t[:, h:QT])
        else:
            evac(a_sb, acc_t, QT)
        a_sbs[si] = a_sb

    def epilogue(si, jq, X, last=False):
        qs = slice(jq * QT, (jq + 1) * QT)
        a_sb = a_sbs.pop(si)
        # denominator rows {16,48,80,112} broadcast to band layout
        rb = m_pool.tile([128, QT], f32, tag="rb", name="rb")
        nc.sync.dma_start(
            out=rb,
            in_=bass.AP(tensor=a_sb.tensor, offset=a_sb.offset + 16 * QT,
                        ap=[[32 * QT, 4], [0, 32], [1, QT]]))
        rec = m_pool.tile([128, QT], f32, tag="rec", name="rec")
        nc.vector.reciprocal_approx_fast(out=rec, in_=rb)
        eng_t["dve"] += QT / 0.96 + 135
        mout = m_pool.tile([128, QT], f32, tag="m", name="m")
        if last:
            h = QT // 2
            nc.gpsimd.tensor_mul(out=mout[:, 0:h], in0=a_sb[:, 0:h],
                                 in1=rec[:, 0:h])
            nc.vector.tensor_mul(out=mout[:, h:QT], in0=a_sb[:, h:QT],
                                 in1=rec[:, h:QT])
        else:
            nc.gpsimd.tensor_mul(out=mout, in0=a_sb, in1=rec)
        # scatter head bands g*32+i -> DRAM channel rows (4X+g)*16+i
        for g in range(4):
            h = 4 * X + g
            eng = nc.sync if g % 2 == 0 else nc.gpsimd
            eng.dma_start(out=out_d[h * DH:(h + 1) * DH, qs],
                          in_=mout[g * 32:g * 32 + DH, :])

    # ---- emission schedule ----
    proj_k(0)
    proj_q(0)
    proj_v()
    streams = [(jq, 0) for jq in range(NJQ)] + \
              [(jq, 1) for jq in range(NJQ)]
    for si, (jq, X) in enumerate(streams):
        lastr = si == len(streams) - 1
        stream(si, jq, X, last=lastr)
        if not lastr:
            pend.append((8, (lambda s: lambda: drain(s))(si)))
            pend.append((11, (lambda s, j, x: lambda: epilogue(s, j, x))(
                si, jq, X)))
        if si == 0:
            proj_k(1)
        elif si == 1:
            proj_q(1)
    drain(len(streams) - 1, split=True)
    epilogue(len(streams) - 1, *streams[-1], last=True)


def _build(n_kv: int, compact: bool) -> "bacc.Bacc":
    Lkv = n_kv * 128
    NX = NXP
    nc = bacc.Bacc("TRN2", target_bir_lowering=False, debug=False,
                   enable_asserts=True, num_devices=B)
    qf_d = nc.dram_tensor("q_full", [D, L], IN_DT, kind="ExternalInput").ap()
    qkv_d = nc.dram_tensor("q_kv", [D, Lkv], IN_DT, kind="ExternalInput").ap()
    wq_d = nc.dram_tensor("wq_sp", [NX, D, D], IN_DT, kind="ExternalInput").ap()
    wk_d = nc.dram_tensor("wk_sp", [NX, D, D], IN_DT, kind="ExternalInput").ap()
    wv_d = nc.dram_tensor("wv_t", [D, D], IN_DT, kind="ExternalInput").ap()
    val_dt = bf16 if compact else f32
    val_shape = [128, n_kv * H] if compact else [128, n_kv]
    val_d = nc.dram_tensor("valid", val_shape, val_dt,
                           kind="ExternalInput").ap()
    out_d = nc.dram_tensor("out", [D, L], f32, kind="ExternalOutput").ap()

    with tile.TileContext(nc) as tc, ExitStack() as ctx:
        _body(ctx, tc, qf_d, qkv_d, wq_d, wk_d, wv_d, val_d, out_d, n_kv,
              compact)
    nc.compile()
    return nc


def _prep_weights(w_mem: np.ndarray, w_query: np.ndarray):
    """Spread head weights into 32-row tile groups (rows 16:32 zero) across
    two planes of 4 head-groups, pre-transposed for use as matmul lhsT.
    Q gets the DH^-0.5 scale."""
    wq_sp = np.zeros((NXP, D, D), np.float32)
    wk_sp = np.zeros((NXP, D, D), np.float32)
    scale = np.float32(DH ** -0.5)
    for X in range(NXP):
        for g in range(4):
            h = 4 * X + g
            wq_sp[X][:, 32 * g:32 * g + DH] = (w_query[DH * h:DH * (h + 1), :] * scale).T
            wk_sp[X][:, 32 * g:32 * g + DH] = w_mem[DH * h:DH * (h + 1), :].T
    wv_t = np.ascontiguousarray(w_mem[D:2 * D, :].T)
    return wq_sp, wk_sp, wv_t


COMPACT_KV = True  # drop masked kv positions host-side (exact: they get a
                   # zero validity column -> contribute 0 to num and denom)


def prepare(queries: np.ndarray, mask: np.ndarray, w_mem: np.ndarray,
            w_query: np.ndarray):
    """Build (compiled program, per-core input maps)."""
    import ml_dtypes
    assert queries.shape == (B, D, L) and mask.shape == (B, L)
    maskf = mask.astype(np.float32)
    kept = [np.nonzero(maskf[b] > 0.0)[0] for b in range(B)]
    if COMPACT_KV and all(len(k) > 0 for k in kept):
        n_kv = max(1, -(-max(len(k) for k in kept) // 128))
        compact = True
    else:
        n_kv = L // 128
        kept = None
        compact = False
    Lkv = n_kv * 128

    key = (n_kv, compact)
    nc = _program_cache.get(key)
    if nc is None:
        nc = _program_cache[key] = _build(n_kv, compact)

    wq_sp, wk_sp, wv_t = _prep_weights(
        w_mem.astype(np.float32), w_query.astype(np.float32))

    in_maps = []
    for b in range(B):
        qb = np.ascontiguousarray(queries[b], dtype=np.float32)
        if kept is not None:
            idx = kept[b]
            qkv = np.zeros((D, Lkv), np.float32)
            qkv[:, :len(idx)] = qb[:, idx]
            val = np.zeros(Lkv, np.float32)
            val[:len(idx)] = 1.0
        else:
            qkv = qb
            val = maskf[b]
        valT = np.ascontiguousarray(val.reshape(n_kv, 128).T)
        in_maps.append({
            "q_full": qb,
            "q_kv": np.ascontiguousarray(qkv),
            "wq_sp": wq_sp,
            "wk_sp": wk_sp,
            "wv_t": wv_t,
            "valid": (np.ascontiguousarray(np.repeat(valT, H, axis=1))
                      .astype(ml_dtypes.bfloat16) if compact else valT),
        })
    return nc, in_maps


def kernel(queries: np.ndarray, mask: np.ndarray, w_mem: np.ndarray,
           w_query: np.ndarray) -> np.ndarray:
    nc, in_maps = prepare(queries, mask, w_mem, w_query)
    res = bass_utils.run_bass_kernel_spmd(nc, in_maps, core_ids=list(range(B)))
    return np.stack([res.results[b]["out"] for b in range(B)]).astype(np.float32)


# revision 15
# speedup vs baseline: 2.0912x; 1.1419x over previous
"""Trainium2 Bass kernel for nn_MultiHeadAttnC (QANet-style self-attention).

Reference computation (per batch b):
    memory = w_mem @ queries[b]          # [2D, L]  (pointwise conv)
    query  = w_query @ queries[b]        # [D, L]
    K, V   = heads of memory             # H=8 heads, DH=16
    Q      = heads of query * DH^-0.5
    S      = Q @ K^T  (masked over kv)   # [H, L, L]
    out[b] = softmax(S) @ V  -> recombined to [D, L]

Strategy (v4):
  - Data parallel: batch b -> NeuronCore b. Weights replicated. No collectives.
  - K-major attention. Heads are split into four 32-row tile groups per
    weight plane so a S^T tile is [128 kv, 2 x 512 q] = 2 PSUM banks: three
    2-bank ring slots double-buffer the exp, and the two leftover banks are
    dedicated AV accumulators. Row-tiled bf16 S matmuls (one bank each)
    stream concurrently (~512 column-cycles per tile).
  - The exp (the roofline: ~27M/core) is SPLIT between the scalar engine
    (native Exp out of PSUM) and the vector engine (single-op Schraudolph:
    i16 = round(x*A + B) == bf16 bits of exp(x); max rel err 3.3%, washes
    out over the ~1600-wide softmax). A greedy time-balancer assigns tiles.
  - AV: col-tiled matmuls (M=17: 16 V channels + validity column for the
    softmax denominator) PSUM-accumulated across ALL kv chunks into the
    stream's accumulator bank (start/stop flags): no per-chunk drains, and
    the ring-slot dependency chain is just S -> exp.
  - Masked kv positions are compacted away host-side (exact: zero validity).
  - Per-q-block epilogue: packed reciprocal on 8 denominator rows, broadcast
    DMA, final multiply on GpSimd, partition-permute DMAs for layout.
"""

import numpy as np
from contextlib import ExitStack

import concourse.bass as bass
import concourse.tile as tile
from concourse import bacc, mybir
from concourse import bass_utils

B, D, L, H, DH = 8, 128, 2048, 8, 16
f32 = mybir.dt.float32
bf16 = mybir.dt.bfloat16
i16 = mybir.dt.int16
f32r = mybir.dt.float32r
IN_DT = f32r
QT = 512             # q columns per stream tile
NJQ = L // QT        # 4
NXP = 2              # weight spread planes (4 head-groups each)

# Schraudolph exp constants for round-to-nearest f32->i16 conversion:
# bf16_bits(exp(x)) ~= round(x * 2^7*log2(e) + (127*2^7 - 5.6))
EXP_A = 184.6649652337873
EXP_B = 16250.4

_program_cache: dict = {}


def _body(ctx, tc, qf_d, qkv_d, wq_d, wk_d, wv_d, val_d, out_d, n_kv, compact):
    nc = tc.nc
    Lkv = n_kv * 128
    Exp = mybir.ActivationFunctionType.Exp
    Copy = mybir.ActivationFunctionType.Copy
    mult, add = mybir.AluOpType.mult, mybir.AluOpType.add
    NX = NXP

    consts = ctx.enter_context(tc.tile_pool(name="consts", bufs=1))

    # ---- input DMAs ----
    wq = consts.tile([D, NX, D], IN_DT, tag="wq")
    wk = consts.tile([D, NX, D], IN_DT, tag="wk")
    for X in range(NX):
        nc.sync.dma_start(out=wk[:, X, :], in_=wk_d[X])
        nc.sync.dma_start(out=wq[:, X, :], in_=wq_d[X])
    wv = consts.tile([D, D], IN_DT, tag="wv")
    nc.sync.dma_start(out=wv, in_=wv_d)
    qkv = consts.tile([D, Lkv], IN_DT, tag="qkv")
    qf = consts.tile([D, L], IN_DT, tag="qf")
    qs_list = [(qkv, qkv_d, c, min(512, Lkv - c))
               for c in range(0, Lkv, 512)]
    qf_list = [(qf, qf_d, j * QT, QT) for j in range(NJQ)]
    # interleave kv/q blocks and alternate queues so both proj streams can
    # start early and neither DMA ring serializes the other
    order = []
    for a, b in zip(qs_list, qf_list + [None] * 9):
        order.append(a)
        if b:
            order.append(b)
    for i, (t, d, c, n) in enumerate(order):
        eng = nc.gpsimd if i % 2 == 0 else nc.sync
        eng.dma_start(out=t[:, c:c + n], in_=d[:, c:c + n])

    q_sp = consts.tile([D, NX, L], bf16, tag="q_sp")
    k_sp = consts.tile([D, NX, Lkv], bf16, tag="k_sp")
    v_sb = consts.tile([128, n_kv, H, DH + 1], bf16, tag="v_sb")
    out_sb = consts.tile([D, L], f32, tag="out_sb")

    if compact:
        # validity (pre-broadcast per head host-side) -> ones column of v_sb
        # via DVE strided write (DMA would clobber neighbors: 2-byte elems)
        val16 = consts.tile([128, n_kv * H], bf16, tag="val16")
        nc.sync.dma_start(out=val16, in_=val_d)
        dst = bass.AP(tensor=v_sb.tensor, offset=v_sb.offset + DH,
                      ap=[[n_kv * H * (DH + 1), 128], [DH + 1, n_kv * H]])
        nc.vector.tensor_copy(out=dst, in_=val16)
    else:
        val = consts.tile([128, n_kv], f32, tag="val")
        nc.gpsimd.dma_start(out=val, in_=val_d)
        ones8 = consts.tile([128, 8], f32, tag="ones8")
        nc.vector.memset(ones8, 1.0)

    # ---- PSUM: 3 duo ring slots (2 banks) + 2 AV accumulator banks ----
    ring = ctx.enter_context(tc.tile_pool(name="ring", bufs=3, space="PSUM"))
    accp = ctx.enter_context(tc.tile_pool(name="accp", bufs=2, space="PSUM"))

    def s_tile():
        return ring.tile([128, 2 * QT], f32, tag="s", name="s")

    # ---- HAM warmup + ACT exp-table prime ----
    warm_in = consts.tile([128, 512], bf16, tag="warm_in")
    nc.vector.memset(warm_in, 0.0)
    wps = accp.tile([128, 512], f32, tag="acc", name="acc")
    for i in range(3):
        nc.tensor.matmul(wps[:, 0:512], lhsT=warm_in[:, 0:128],
                         rhs=warm_in, start=True, stop=True)
    p_warm = consts.tile([128, 128], bf16, tag="p_warm")
    nc.scalar.activation(out=p_warm, in_=warm_in[:, 0:128], func=Exp)

    # ---- projections: PSUM->SBUF evacuation goes to the less-loaded of
    # scalar/vector, tracked in the same ledger the exp balancer uses ----
    eng_t = {"act": 0.0, "dve": 6000.0}

    def evac(dst_ap, src_ap):
        n = dst_ap.shape[-1] if hasattr(dst_ap, "shape") else 1024
        c_act = eng_t["act"] + n / 1.2 + 160
        c_dve = eng_t["dve"] + n / 0.96 + 160
        if c_act <= c_dve:
            eng_t["act"] = c_act
            nc.scalar.activation(out=dst_ap, in_=src_ap, func=Copy)
        else:
            eng_t["dve"] = c_dve
            nc.vector.tensor_copy(out=dst_ap, in_=src_ap)

    def proj_k(X):
        col = 0
        while col < Lkv:
            n = min(2 * QT, Lkv - col)
            ps = s_tile()
            for off in range(0, n, 512):
                m = min(512, n - off)
                nc.tensor.matmul(ps[:, off:off + m], lhsT=wk[:, X, :],
                                 rhs=qkv[:, col + off:col + off + m],
                                 start=True, stop=True)
            evac(k_sp[:, X, col:col + n], ps[:, 0:n])
            col += n

    def proj_q(X):
        col = 0
        while col < L:
            n = min(2 * QT, L - col)
            ps = s_tile()
            for off in range(0, n, 512):
                m = min(512, n - off)
                nc.tensor.matmul(ps[:, off:off + m], lhsT=wq[:, X, :],
                                 rhs=qf[:, col + off:col + off + m],
                                 start=True, stop=True)
            evac(q_sp[:, X, col:col + n], ps[:, 0:n])
            col += n

    def proj_v():
        for c in range(n_kv):
            vp = accp.tile([128, 512], f32, tag="acc", name="acc")
            nc.tensor.matmul(vp[:, 0:D], lhsT=qkv[:, c * 128:(c + 1) * 128],
                             rhs=wv, start=True, stop=True)
            if compact:
                nc.vector.tensor_copy(
                    out=v_sb[:, c, :, 0:DH],
                    in_=vp[:, 0:D].rearrange("p (h x) -> p h x", x=DH))
            else:
                nc.vector.tensor_scalar_mul(
                    v_sb[:, c, :, 0:DH],
                    vp[:, 0:D].rearrange("p (h x) -> p h x", x=DH),
                    val[:, c:c + 1])
                nc.vector.tensor_scalar_mul(
                    v_sb[:, c, :, DH:DH + 1],
                    ones8.rearrange("p (h x) -> p h x", x=1),
                    val[:, c:c + 1])

    proj_k(0)
    proj_q(0)
    proj_v()

    # ---- attention ----
    p_act = ctx.enter_context(tc.tile_pool(name="p_act", bufs=7))
    p_dve = ctx.enter_context(tc.tile_pool(name="p_dve", bufs=6))
    a_pool = ctx.enter_context(tc.tile_pool(name="a_pool", bufs=4))
    misc = ctx.enter_context(tc.tile_pool(name="misc", bufs=2))

    drains = {}

    def stream_pair(jq, X):
        """Both head-pair streams (j=0: groups 0,1 / j=1: groups 2,3) of one
        (jq, X) advance chunk-by-chunk together: their S duos use disjoint
        row-groups and their AV duos disjoint col-groups, so the PE streams
        4 matmuls concurrently (quad efficiency) while each exp tile stays
        2 banks (duo) for the 3-slot ring + 2 accumulator-bank layout."""
        n = 2 * QT
        qs = slice(jq * QT, (jq + 1) * QT)
        accs = {j: accp.tile([128, QT], f32, tag="acc", name="acc")
                for j in (0, 1)}

        def s_duo(c, j):
            ck = slice(c * 128, (c + 1) * 128)
            sp = s_tile()
            for gi in range(2):
                g = 2 * j + gi
                nc.tensor.matmul(
                    sp[:, gi * QT:(gi + 1) * QT],
                    lhsT=k_sp[g * 32:(g + 1) * 32, X, ck],
                    rhs=q_sp[g * 32:(g + 1) * 32, X, qs],
                    start=True, stop=True, tile_position=(g * 32, 0))
            return sp

        sps, ps, avq = {}, {}, []

        def emit_av(c):
            st, en = (c == 0), (c == n_kv - 1)
            for j in (0, 1):
                rhs_p = ps.pop((c, j))
                for gi in range(2):
                    g = 2 * j + gi
                    nc.tensor.matmul(
                        accs[j][g * 32:g * 32 + DH + 1, :],
                        lhsT=v_sb[:, c, 4 * X + g, :],
                        rhs=rhs_p[:, gi * QT:(gi + 1) * QT],
                        start=st, stop=en, tile_position=(0, g * 32))

        for j in (0, 1):
            sps[(0, j)] = s_duo(0, j)
        for c in range(n_kv):
            for j in (0, 1):
                sp = sps.pop((c, j))
                c_act = eng_t["act"] + n / 1.2 + 290
                c_dve = eng_t["dve"] + (n / 0.96 + 160) * 1.1
                if c_act <= c_dve:
                    eng_t["act"] = c_act
                    p = p_act.tile([128, 2 * QT], bf16, tag="p")
                    nc.scalar.activation(out=p, in_=sp, func=Exp)
                    ps[(c, j)] = p
                else:
                    eng_t["dve"] = c_dve
                    p16 = p_dve.tile([128, 2 * QT], i16, tag="p16")
                    nc.vector.tensor_scalar(out=p16, in0=sp,
                                            scalar1=EXP_A, scalar2=EXP_B,
                                            op0=mult, op1=add)
                    ps[(c, j)] = p16.bitcast(bf16)
            if c + 1 < n_kv:
                for j in (0, 1):
                    sps[(c + 1, j)] = s_duo(c + 1, j)
            avq.append(c)
            if len(avq) > 3:
                emit_av(avq.pop(0))
        while avq:
            emit_av(avq.pop(0))
        last = (jq == NJQ - 1 and X == 1)
        for j in (0, 1):
            a_sb = a_pool.tile([64, QT], f32, tag=f"a{X}{j}", name="a")
            base = 64 * j
            if last and j == 0:
                # final pair: ACT is idle (exps done) - parallelize the drains
                nc.scalar.activation(out=a_sb, in_=accs[j][base:base + 64, :],
                                     func=Copy)
            else:
                nc.vector.tensor_copy(out=a_sb, in_=accs[j][base:base + 64, :])
                eng_t["dve"] += QT / 0.96 + 160
            drains[(jq, X, j)] = a_sb

    dmaq = [nc.gpsimd, nc.sync]

    def epilogue(jq):
        qs = slice(jq * QT, (jq + 1) * QT)
        pk = misc.tile([8, QT], f32, tag="pk")
        qi = [0]

        def gdma(out, in_):
            qi[0] = (qi[0] + 1) % len(dmaq)
            dmaq[qi[0]].dma_start(out=out, in_=in_)

        for X in range(2):
            for j in range(2):
                a_sb = drains[(jq, X, j)]
                for gi in range(2):
                    h = 4 * X + 2 * j + gi
                    gdma(pk[h:h + 1, :],
                         a_sb[gi * 32 + DH:gi * 32 + DH + 1, :])
        rec = misc.tile([8, QT], f32, tag="rec")
        nc.vector.reciprocal_approx_fast(out=rec, in_=pk)
        eng_t["dve"] += 2 * QT / 0.96 + 160
        rb = misc.tile([128, QT], f32, tag="rb")
        nc.sync.dma_start(
            out=rb,
            in_=bass.AP(tensor=rec.tensor, offset=rec.offset,
                        ap=[[QT, 8], [0, DH], [1, QT]]))
        xt = misc.tile([128, QT], f32, tag="xt")
        for X in range(2):
            for j in range(2):
                a_sb = drains[(jq, X, j)]
                for gi in range(2):
                    h = 4 * X + 2 * j + gi
                    gdma(xt[h * DH:(h + 1) * DH, :],
                         a_sb[gi * 32:gi * 32 + DH, :])
        if jq == NJQ - 1:
            # final epilogue is the un-hidden tail: split the multiply
            h = QT // 2
            nc.gpsimd.tensor_mul(out=out_sb[:, jq * QT:jq * QT + h],
                                 in0=xt[:, 0:h], in1=rb[:, 0:h])
            nc.vector.tensor_mul(out=out_sb[:, jq * QT + h:(jq + 1) * QT],
                                 in0=xt[:, h:QT], in1=rb[:, h:QT])
        else:
            nc.gpsimd.tensor_mul(out=out_sb[:, qs], in0=xt, in1=rb)
        nc.sync.dma_start(out=out_d[:, qs], in_=out_sb[:, qs])

    # interleave X0/X1 phases two pairs apart: epilogues spread evenly
    # through the run instead of bursting DVE in the second half
    stream_pair(0, 0)
    proj_k(1)
    stream_pair(1, 0)
    proj_q(1)
    for jq in range(NJQ):
        if jq + 2 < NJQ:
            stream_pair(jq + 2, 0)
        stream_pair(jq, 1)
        epilogue(jq)


def _build(n_kv: int, compact: bool) -> "bacc.Bacc":
    Lkv = n_kv * 128
    NX = NXP
    nc = bacc.Bacc("TRN2", target_bir_lowering=False, debug=False,
                   enable_asserts=True, num_devices=B)
    qf_d = nc.dram_tensor("q_full", [D, L], IN_DT, kind="ExternalInput").ap()
    qkv_d = nc.dram_tensor("q_kv", [D, Lkv], IN_DT, kind="ExternalInput").ap()
    wq_d = nc.dram_tensor("wq_sp", [NX, D, D], IN_DT, kind="ExternalInput").ap()
    wk_d = nc.dram_tensor("wk_sp", [NX, D, D], IN_DT, kind="ExternalInput").ap()
    wv_d = nc.dram_tensor("wv_t", [D, D], IN_DT, kind="ExternalInput").ap()
    val_dt = bf16 if compact else f32
    val_shape = [128, n_kv * H] if compact else [128, n_kv]
    val_d = nc.dram_tensor("valid", val_shape, val_dt,
                           kind="ExternalInput").ap()
    out_d = nc.dram_tensor("out", [D, L], f32, kind="ExternalOutput").ap()

    with tile.TileContext(nc) as tc, ExitStack() as ctx:
        _body(ctx, tc, qf_d, qkv_d, wq_d, wk_d, wv_d, val_d, out_d, n_kv,
              compact)
    nc.compile()
    return nc


def _prep_weights(w_mem: np.ndarray, w_query: np.ndarray):
    """Spread head weights into 32-row tile groups (rows 16:32 zero) across
    two planes of 4 head-groups, pre-transposed for use as matmul lhsT.
    Q gets the DH^-0.5 scale."""
    wq_sp = np.zeros((NXP, D, D), np.float32)
    wk_sp = np.zeros((NXP, D, D), np.float32)
    scale = np.float32(DH ** -0.5)
    for X in range(NXP):
        for g in range(4):
            h = 4 * X + g
            wq_sp[X][:, 32 * g:32 * g + DH] = (w_query[DH * h:DH * (h + 1), :] * scale).T
            wk_sp[X][:, 32 * g:32 * g + DH] = w_mem[DH * h:DH * (h + 1), :].T
    wv_t = np.ascontiguousarray(w_mem[D:2 * D, :].T)
    return wq_sp, wk_sp, wv_t


COMPACT_KV = True  # drop masked kv positions host-side (exact: they get a
                   # zero validity column -> contribute 0 to num and denom)


def prepare(queries: np.ndarray, mask: np.ndarray, w_mem: np.ndarray,
            w_query: np.ndarray):
    """Build (compiled program, per-core input maps)."""
    import ml_dtypes
    assert queries.shape == (B, D, L) and mask.shape == (B, L)
    maskf = mask.astype(np.float32)
    kept = [np.nonzero(maskf[b] > 0.0)[0] for b in range(B)]
    if COMPACT_KV and all(len(k) > 0 for k in kept):
        n_kv = max(1, -(-max(len(k) for k in kept) // 128))
        compact = True
    else:
        n_kv = L // 128
        kept = None
        compact = False
    Lkv = n_kv * 128

    key = (n_kv, compact)
    nc = _program_cache.get(key)
    if nc is None:
        nc = _program_cache[key] = _build(n_kv, compact)

    wq_sp, wk_sp, wv_t = _prep_weights(
        w_mem.astype(np.float32), w_query.astype(np.float32))

    in_maps = []
    for b in range(B):
        qb = np.ascontiguousarray(queries[b], dtype=np.float32)
        if kept is not None:
            idx = kept[b]
            qkv = np.zeros((D, Lkv), np.float32)
            qkv[:, :len(idx)] = qb[:, idx]
            val = np.zeros(Lkv, np.float32)
            val[:len(idx)] = 1.0
        else:
            qkv = qb
            val = maskf[b]
        valT = np.ascontiguousarray(val.reshape(n_kv, 128).T)
        in_maps.append({
            "q_full": qb,
            "q_kv": np.ascontiguousarray(qkv),
            "wq_sp": wq_sp,
            "wk_sp": wk_sp,
            "wv_t": wv_t,
            "valid": (np.ascontiguousarray(np.repeat(valT, H, axis=1))
                      .astype(ml_dtypes.bfloat16) if compact else valT),
        })
    return nc, in_maps


def kernel(queries: np.ndarray, mask: np.ndarray, w_mem: np.ndarray,
           w_query: np.ndarray) -> np.ndarray:
    nc, in_maps = prepare(queries, mask, w_mem, w_query)
    res = bass_utils.run_bass_kernel_spmd(nc, in_maps, core_ids=list(range(B)))
    return np.stack([res.results[b]["out"] for b in range(B)]).astype(np.float32)
